# revision 67
# baseline (speedup 1.0000x reference)
"""DiT block with block-diffusion sparse attention on 8 Trainium2 NeuronCores.

v4 strategy (evolution of v3; see kernel_v3.py):
  - Clean-half-first software pipeline: per-512-token-quarter LN1 stats
    (bn_stats on fp8 token-major x) feed QKV chunks as they complete; the
    clean half (tokens 1024:2048) goes first so attention chunks c=2,3 can
    run interleaved with the noisy half's QKV matmuls on the PE.
  - rstd is NOT folded into the rope tables for k/v. Instead:
      k: per-partition scale at the softmax EXP (scores are [k, q] so
         rstd_k is a per-partition column; q's rstd*0.125 is folded into
         the q rope tables via one row-broadcast per quarter).
      v: per-partition scale on the V-transpose PSUM evacuation.
    This removes 2/3 of the rstd broadcast/fold work.
  - Stats avoid the DRAM bounce: per-tile [128,4] (negmu, sd, rstd/8) PE
    transposes into rows4[4, S]; rstd also kept token-tile-major in
    rstdc[128,16] for the exp/v scales.
  - rotate-half swap copies on GpSimd; attention mask multiplies on DVE
    (idle during attention); softmax reciprocal via reciprocal_approx_fast.
  - Softmax denominators/normalization per half: the clean half is
    normalized and staged to DRAM while the noisy half's attention still
    runs; only the noisy half's norm is exposed before the AllToAll.
  - Single consolidated DMAs with >=1KB descriptor lines; issues spread
    over the sync/tensor/gpsimd queues; w1/w2 prefetch gated behind the
    x loads via gpsimd program order.
  - attn_out uses full 128-partition stationary tiles (2 source cores per
    DoubleRow pair) - 2x fewer PE cycles than v3's 64-partition layout.
  - LN2 sum/sumsq matmuls interleaved with the attn_out chains; LN2 row
    broadcasts via stride-0 DMA instead of PE rank-1 + ACT evac.
"""

import os
import numpy as np
import ml_dtypes

import concourse.bass as bass
import concourse.tile as tile
from concourse import bacc, mybir
from concourse.bass_utils import run_bass_kernel_spmd
from concourse.masks import make_identity

bf16 = ml_dtypes.bfloat16
fp8 = ml_dtypes.float8_e4m3
FP = mybir.dt.float32
BF = mybir.dt.bfloat16
F8 = mybir.dt.float8e4
AF = mybir.ActivationFunctionType
ALU = mybir.AluOpType
DR = mybir.MatmulPerfMode.DoubleRow
WQSCALE = 64.0
WAOSCALE = 64.0
W1SCALE = 64.0
W2SCALE = 128.0

NCORES = 8
S, N, D, H, HD, BS, COND = 2048, 1024, 1024, 16, 64, 16, 128
TOK = S // NCORES  # 256 tokens per core after A2A

QORDER = [2, 3, 0, 1]  # clean half first


def _attn_schedule():
    """Per q-chunk list of (ktile, col0, col1, mask) in S^T orientation."""
    sched = []
    for c in range(4):
        items = []
        if c < 2:  # noisy q chunk
            for j in range(4 * c + 4):  # clean k tiles, bq > bk
                js = j - 4 * c
                if js < 0:
                    items.append((8 + j, 0, 512, None))
                else:
                    items.append((8 + j, 128 * js, 512, "strict"))
            for s in range(4):  # own-block diagonal (noisy k)
                items.append((4 * c + s, 128 * s, 128 * s + 128, "diag"))
        else:  # clean q chunk, bq >= bk
            cq = c - 2
            for j in range(4 * cq + 4):
                js = j - 4 * cq
                if js < 0:
                    items.append((8 + j, 0, 512, None))
                else:
                    items.append((8 + j, 128 * js, 512, "incl"))
        assert items[0][1] == 0 and items[0][2] == 512
        sched.append(items)
    return sched


MASK_OFF = {"diag": 0, "strict": 128, "incl": 256}


def build_program(single=False, dbg=False):
    nc = bacc.Bacc(
        "TRN2", target_bir_lowering=False, debug=False,
        enable_asserts=False, num_devices=1 if single else NCORES,
    )

    def din(name, shape, dt=BF):
        return nc.dram_tensor(name, shape, dt, kind="ExternalInput").ap()

    rows2_d = din("rows2", [2, S])                    # negmu | sd (host)
    r8_d = din("r8row", [1, S])                       # rstd/8 (host)
    rstdc_d = din("rstdc", [128, 16], FP)             # rstd token-tile cols
    xT_d = din("xT", [4, 4, 128, 2, 512], F8)         # (n, j, p, i, t)
    trig_d = din("trig", [2, 128, 1024])              # cos128 | sin128(signed)
    mask01_d = din("mask01", [128, 384])              # diag|strict|incl
    wqkv_d = din("wqkvT", [128, 3, 4, 2, 128], F8)    # (p, s, j, i, c) scaled
    ub_d = din("ubrow", [2, 384])                     # (u; b) per-core slice
    xsT_d = din("xsliceT", [128, 8, 256])             # residual (p, k, t)
    wao_d = din("waoT", [128, 8, 4, 2, 128], F8)      # (p=(i,hd), m, j, i2, c)
    w1_d = din("w1T", [128, 8, 4, 4, 2, 128], F8)     # (p, g, mi, j, i, c)
    w2_d = din("w2T", [128, 8, 16, 2, 128], F8)       # (p, m, j, i, c)
    smallc_d = din("smallc", [128, 64], FP)           # gmsa|gmlp|b1'|b2|gb2
    out_d = nc.dram_tensor("out", [128, 8, TOK], FP, kind="ExternalOutput").ap()
    dbg_d = (nc.dram_tensor("dbg", [8, 128, S], BF,
                            kind="ExternalOutput").ap() if dbg else None)
    dbgf8_d = (nc.dram_tensor("dbgf8", [3, 128, S], F8,
                              kind="ExternalOutput").ap() if dbg else None)
    dbg32_d = (nc.dram_tensor("dbg32", [2, 128, S], FP,
                              kind="ExternalOutput").ap() if dbg else None)

    sched = _attn_schedule()

    with tile.TileContext(nc) as tc:
        with tc.tile_pool(name="const", bufs=1) as const, \
             tc.tile_pool(name="dram", bufs=1, space="DRAM") as dram, \
             tc.tile_pool(name="qkvr", bufs=1) as qkvr, \
             tc.tile_pool(name="vaugp", bufs=1) as vaugp, \
             tc.tile_pool(name="x2p", bufs=1) as x2p, \
             tc.tile_pool(name="gp", bufs=1) as gp:

            # ---------------- DMA issues ------------------------------
            # long-lived weight pools first (pool releases are LIFO)
            waop = tc.alloc_tile_pool(name="waop", bufs=1)
            w1p = tc.alloc_tile_pool(name="w1p", bufs=1)
            w2p = tc.alloc_tile_pool(name="w2p", bufs=1)
            xTp = tc.alloc_tile_pool(name="xTp", bufs=1)
            xTn_sb = {}
            for q in QORDER:
                xTn_sb[q] = xTp.tile([128, 4, 2, 512], F8, name=f"xTn{q}")

            ub_sb = const.tile([2, 384], BF)
            rows4 = const.tile([4, S], BF)        # negmu | sd (host)
            r8row = const.tile([1, S], BF)        # rstd/8 at partition 0
            rtmp = const.tile([1, 512], BF)       # bcast staging row
            rstdc = const.tile([128, 16], FP)     # per token-tile rstd col
            trig_sb = const.tile([128, 2, 1024], BF)
            mask_sb = const.tile([128, 384], BF)
            smallc = const.tile([128, 64], FP)
            xsT = const.tile([128, 8, 256], BF)

            # sync queue: host stats rows first, then rope tables
            nc.sync.dma_start(out=rows4[0:2, :], in_=rows2_d)
            nc.sync.dma_start(out=r8row, in_=r8_d)
            nc.sync.dma_start(out=rstdc, in_=rstdc_d)
            nc.sync.dma_start(out=ub_sb, in_=ub_d)
            nc.sync.dma_start(out=trig_sb,
                              in_=trig_d.rearrange("v p t -> p v t"))

            # scalar queue: qkv weights + xT chunks (hot path first)
            wq_sb = const.tile([128, 3, 4, 2, 128], F8)
            nc.scalar.dma_start(
                out=wq_sb, in_=wqkv_d)
            nc.scalar.dma_start(out=xTn_sb[2], in_=xT_d[2].rearrange("j p i t -> p j i t"))
            nc.scalar.dma_start(out=xTn_sb[3], in_=xT_d[3].rearrange("j p i t -> p j i t"))
            nc.scalar.dma_start(out=xTn_sb[0], in_=xT_d[0].rearrange("j p i t -> p j i t"))
            nc.scalar.dma_start(out=xTn_sb[1], in_=xT_d[1].rearrange("j p i t -> p j i t"))
            # cold constants on sync after the hot issues
            nc.sync.dma_start(out=mask_sb, in_=mask01_d)
            nc.sync.dma_start(out=smallc, in_=smallc_d)
            nc.sync.dma_start(out=xsT, in_=xsT_d)

            gmsa_sb = smallc[:, 0:8]
            gmlp_sb = smallc[:, 8:16]
            b1_sb = smallc[:, 16:48]
            b2_sb = smallc[:, 48:56]
            gb2_sb = smallc[:, 56:64]
            cosT = trig_sb[:, 0, :]
            sinT = trig_sb[:, 1, :]

            ones_sb = const.tile([128, 1], BF)
            nc.vector.memset(ones_sb, 1.0)
            eps128 = const.tile([128, 1], FP)
            nc.vector.memset(eps128, 1e-5)
            eps1 = const.tile([1, 1], FP)
            nc.vector.memset(eps1, 1e-5)
            ident_f = const.tile([128, 128], FP)
            make_identity(nc, ident_f)
            ident_b = const.tile([128, 128], BF)
            nc.vector.tensor_copy(out=ident_b, in_=ident_f)


            qT = qkvr.tile([128, S], BF)
            kT = qkvr.tile([128, S], BF)
            vT = qkvr.tile([128, S], BF)
            qkv_dst = [qT, kT, vT]
            vaug = [vaugp.tile([128, 130], BF, name=f"vaug{kt}")
                    for kt in range(16)]

            onorm = [qkvr.tile([128, N], F8, name=f"onorm{hh}")
                     for hh in range(2)]
            obounce = dram.tile([NCORES, 128, TOK], F8)
            orecvb = dram.tile([NCORES, 128, TOK], F8)
            orecv2 = x2p.tile([128, 8, TOK], F8)
            ounp = tc.alloc_tile_pool(name="ounp", bufs=4)
            o_un = {}
            den4 = [qkvr.tile([128, 512], FP, name=f"den{hh}")
                    for hh in range(2)]
            recip4 = qkvr.tile([128, 512], FP, name="recip4")
            for hh in range(2):
                nc.vector.memset(den4[hh], 1.0)

            # attn_out weights prefetch (gated on gpsimd order below)
            wao_sb = waop.tile([128, 8 * 4 * 2 * 128], F8, name="wao")
            w1_sb = w1p.tile([128, 8 * 4 * 4 * 2 * 128], F8, name="w1")
            w2_sb = w2p.tile([128, 8 * 16 * 2 * 128], F8, name="w2")
            w2r = w2_sb.rearrange("p (m j i c) -> p m j i c", m=8, j=16,
                                  i=2, c=128)
            waor = wao_sb.rearrange("p (m j i c) -> p m j i c",
                                    m=8, j=4, i=2, c=128)
            w1r = w1_sb.rearrange("p (g mi j i c) -> p g mi j i c",
                                  g=8, mi=4, j=4, i=2, c=128)

            qtab = tc.alloc_tile_pool(name="qtab", bufs=2)
            ropep = tc.alloc_tile_pool(name="ropep", bufs=3)
            rbp = tc.alloc_tile_pool(name="rbp", bufs=1)
            nrmp = tc.alloc_tile_pool(name="nrmp", bufs=2)
            qtabs = {}
            mmps = tc.alloc_tile_pool(name="mmps", bufs=2, space="PSUM")
            sps = tc.alloc_tile_pool(name="sps", bufs=2, space="PSUM")
            ops = tc.alloc_tile_pool(name="ops", bufs=2, space="PSUM")

            wqr = wq_sb

            # ---------------- phase builders --------------------------
            def fold(q):
                nsl = slice(512 * q, 512 * q + 512)
                nmod = slice(512 * (q % 2), 512 * (q % 2) + 512)
                rb = rbp.tile([128, 512], BF, tag="rb")
                nc.gpsimd.partition_broadcast(rb, r8row[:, nsl])
                cq = qtab.tile([128, 512], BF, tag="cosq", name=f"cosq{q}")
                sq = qtab.tile([128, 512], BF, tag="sinq", name=f"sinq{q}")
                nc.vector.tensor_mul(cq, cosT[:, nmod], rb)
                nc.vector.tensor_mul(sq, sinT[:, nmod], rb)
                qtabs[q] = (cq, sq)

            def qkv_m(n, m):
                nsl = slice(512 * n, 512 * n + 512)
                nmod = slice(512 * (n % 2), 512 * (n % 2) + 512)
                if True:
                    ps = mmps.tile([128, 512], FP, tag="mm", name="qkvps")
                    for j in range(4):
                        nc.tensor.matmul(
                            ps, wqr[:, m, j],
                            xTn_sb[n][:, j], perf_mode=DR,
                            start=(j == 0), stop=False,
                            skip_group_check=True)
                    nc.tensor.matmul(
                        ps, ub_sb[:, 128 * m:128 * m + 128],
                        rows4[0:2, nsl], start=False, stop=True,
                        skip_group_check=True)
                    pb = ropep.tile([128, 512], BF, tag="pb")
                    nc.scalar.copy(out=pb, in_=ps)
                    pbs = ropep.tile([128, 512], BF, tag="pbs")
                    for h in range(2):
                        r = 64 * h
                        nc.vector.tensor_copy(
                            out=pbs[r:r + 32, :], in_=pb[r + 32:r + 64, :])
                        nc.vector.tensor_copy(
                            out=pbs[r + 32:r + 64, :], in_=pb[r:r + 32, :])
                    ca = qtabs[n][0] if m == 0 else cosT[:, nmod]
                    sa = qtabs[n][1] if m == 0 else sinT[:, nmod]
                    t1 = ropep.tile([128, 512], BF, tag="t1")
                    nc.vector.tensor_mul(t1, pb, ca)
                    nc.vector.tensor_mul(pbs, pbs, sa)
                    nc.vector.tensor_add(qkv_dst[m][:, nsl], t1, pbs)

            def qkv(n):
                for m in (1, 2, 0):  # q last: more slack for the fold
                    qkv_m(n, m)

            def vaug_one(kt):
                ps = mmps.tile([128, 512], FP, tag="mm", name="vtps")
                psb = ps.bitcast(BF)
                nc.tensor.transpose(
                    psb[:, 0:128], vT[:, 128 * kt:128 * kt + 128],
                    ident_b)
                va = vaug[kt]
                nc.vector.memset(va[:, 64:65], 1.0)
                nc.vector.memset(va[:, 129:130], 1.0)
                nc.scalar.activation(
                    out=va[:, 0:130].rearrange(
                        "p (h y) -> p h y", y=65)[:, :, 0:64],
                    in_=psb[:, 0:128].rearrange("p (h d) -> p h d", d=64),
                    func=AF.Copy, scale=rstdc[:, kt:kt + 1])

            def vaug_blk(n):
                for kt in range(4 * n, 4 * n + 4):
                    ps = mmps.tile([128, 512], FP, tag="mm", name="vtps")
                    psb = ps.bitcast(BF)
                    nc.tensor.transpose(
                        psb[:, 0:128], vT[:, 128 * kt:128 * kt + 128],
                        ident_b)
                    va = vaug[kt]
                    nc.vector.memset(va[:, 64:65], 1.0)
                    nc.vector.memset(va[:, 129:130], 1.0)
                    # v gets its token's rstd here (per-partition scale)
                    nc.scalar.activation(
                        out=va[:, 0:130].rearrange(
                            "p (h y) -> p h y", y=65)[:, :, 0:64],
                        in_=psb[:, 0:128].rearrange("p (h d) -> p h d", d=64),
                        func=AF.Copy, scale=rstdc[:, kt:kt + 1])

            def attn(c, fillers=None):
                fillers = list(fillers or [])
                items = sched[c]
                nit = len(items)
                o_ps = {h: ops.tile([65, 512], FP, tag="ops",
                                    name=f"ops{c}_{h}")
                        for h in range(2)}
                q0 = 512 * c
                s_tiles = {}

                def score(idx):
                    kt, c0, c1, mk = items[idx]
                    w = c1 - c0
                    s_ps = sps.tile([128, 2, 512], FP, tag="sps")
                    s_tiles[idx] = s_ps
                    for h in range(2):
                        nc.tensor.matmul(
                            s_ps[:, h, 0:w],
                            kT[64 * h:64 * h + 64,
                               128 * kt:128 * kt + 128],
                            qT[64 * h:64 * h + 64, q0 + c0:q0 + c1],
                            start=True, stop=True,
                            skip_group_check=True)

                def finish(idx):
                    kt, c0, c1, mk = items[idx]
                    w = c1 - c0
                    s_ps = s_tiles.pop(idx)
                    p_sb = ropep.tile([128, 2, 512], BF, tag="pt")
                    sc = rstdc[:, kt:kt + 1]
                    if w == 512:
                        nc.scalar.activation(out=p_sb[:, :, :],
                                             in_=s_ps[:, :, :],
                                             func=AF.Exp, scale=sc)
                    else:
                        for h in range(2):
                            nc.scalar.activation(
                                out=p_sb[:, h, 0:w],
                                in_=s_ps[:, h, 0:w],
                                func=AF.Exp, scale=sc)
                    if mk is not None:
                        mo = MASK_OFF[mk]
                        for h in range(2):
                            nc.vector.tensor_mul(
                                p_sb[:, h, 0:128], p_sb[:, h, 0:128],
                                mask_sb[:, mo:mo + 128])
                    for h in range(2):
                        nc.tensor.matmul(
                            o_ps[h][:, c0:c1],
                            vaug[kt][:, 65 * h:65 * h + 65],
                            p_sb[:, h, 0:w], start=(idx == 0),
                            stop=(idx == nit - 1),
                            skip_group_check=True)

                score(0)
                for idx in range(1, nit):
                    score(idx)
                    if fillers:
                        fillers.pop(0)()
                    finish(idx - 1)
                finish(nit - 1)
                while fillers:
                    fillers.pop(0)()
                hh = 1 if c >= 2 else 0
                for h in range(2):
                    k = 2 * c + h
                    r = 32 * (k % 4)
                    o_un[k] = ounp.tile([64, 512], BF, tag="oun",
                                        name=f"oun{k}")
                    nc.scalar.copy(out=o_un[k], in_=o_ps[h][0:64, :])
                    nc.scalar.copy(out=den4[hh][r:r + 1, :],
                                   in_=o_ps[h][64:65, :])

            def norm_chunk(c):
                hh = 1 if c >= 2 else 0
                rc4 = nrmp.tile([128, 512], FP, tag="rc4")
                nc.vector.reciprocal_approx_fast(out=rc4, in_=den4[hh])
                for h in range(2):
                    k = 2 * c + h
                    r = 32 * (k % 4)
                    rt = nrmp.tile([1, 512], BF, tag="rt")
                    nc.vector.tensor_copy(out=rt, in_=rc4[r:r + 1, :])
                    rbc = nrmp.tile([64, 512], BF, tag="rbc")
                    nc.gpsimd.partition_broadcast(rbc, rt)
                    nc.vector.tensor_mul(
                        onorm[hh][64 * h:64 * h + 64,
                                  (512 * c) % N:(512 * c) % N + 512],
                        o_un[k], rbc)

            def stage_half(hh):
                nc.sync.dma_start(
                    out=obounce[4 * hh:4 * hh + 4].rearrange(
                        "j p t -> p j t"),
                    in_=onorm[hh].rearrange("p (j t) -> p j t", t=TOK))

            # ---------------- emission order --------------------------
            fold(2)
            fold(3)
            qkv(2)
            vaug_blk(2)
            # weight prefetch gated on the first vaug block (~20us local):
            # keeps the 9.4MB weight stream out of every core's x-load window
            gate = rbp.tile([1, 1], BF, tag="gate")
            nc.gpsimd.tensor_copy(out=gate, in_=vaug[11][0:1, 0:1])
            nc.gpsimd.dma_start(out=wao_sb.rearrange(
                "p (m j i c) -> p m j i c", m=8, j=4, i=2, c=128),
                in_=wao_d)
            nc.gpsimd.dma_start(out=w1_sb.rearrange(
                "p (g mi j i c) -> p g mi j i c", g=8, mi=4, j=4, i=2,
                c=128), in_=w1_d)
            nc.gpsimd.dma_start(out=w2_sb.rearrange(
                "p (m j i c) -> p m j i c", m=8, j=16, i=2, c=128),
                in_=w2_d)
            qkv(3)
            vaug_blk(3)
            fold(0)
            fold(1)
            attn(2, [lambda m=m: qkv_m(0, m) for m in (1, 2, 0)])
            attn(3, [lambda kt=kt: vaug_one(kt) for kt in range(0, 4)]
                 + [lambda m=m: qkv_m(1, m) for m in (1, 2, 0)])
            attn(0, [lambda kt=kt: vaug_one(kt) for kt in range(4, 8)]
                 + [lambda c=c: norm_chunk(c) for c in (2, 3)]
                 + [lambda: stage_half(1)])
            attn(1, [lambda: norm_chunk(0)])
            norm_chunk(1)
            stage_half(0)

            if dbg_d is not None:
                nc.sync.dma_start(out=dbg_d[0], in_=qT)
                nc.sync.dma_start(out=dbg_d[1], in_=kT)
                nc.sync.dma_start(out=dbg_d[2], in_=vT)
                for hh in range(2):
                    nc.sync.dma_start(out=dbgf8_d[0][:, N * hh:N * hh + N],
                                      in_=onorm[hh])
                    nc.sync.dma_start(
                        out=dbg32_d[0][:, 512 * hh:512 * hh + 512],
                        in_=den4[hh])
                nc.sync.dma_start(out=dbg32_d[0][:, 1024:1040], in_=rstdc)
                nc.sync.dma_start(out=dbg_d[5][0:4, :], in_=rows4)
                nc.sync.dma_start(out=dbg_d[6][0:1, :], in_=r8row)

            if single:
                nc.sync.dma_start(out=orecvb[:], in_=obounce[:])
            else:
                nc.gpsimd.collective_compute(
                    "AllToAll", ALU.bypass,
                    replica_groups=[list(range(NCORES))],
                    ins=[obounce.opt()], outs=[orecvb.opt()])
            for jp in range(4):
                nc.sync.dma_start(
                    out=orecv2[:, 2 * jp:2 * jp + 2, :],
                    in_=orecvb[2 * jp:2 * jp + 2].rearrange(
                        "k r t -> r k t"))

            ops.release()
            sps.release()
            mmps.release()
            nrmp.release()
            rbp.release()
            ropep.release()
            qtab.release()
            ounp.release()
            xTp.release()

            # ---------------- phase 4: attn_out + residual + LN2 stats -
            x2T = [x2p.tile([128, TOK], FP, name=f"x2T{m}") for m in range(8)]
            x2b = [x2p.tile([128, TOK], BF, name=f"x2b{m}") for m in range(8)]
            sqb = [x2p.tile([128, TOK], BF, name=f"sqb{m}") for m in range(8)]
            h2dr = [x2p.tile([128, 2, TOK], F8, name=f"h2dr{j}")
                    for j in range(4)]
            with tc.tile_pool(name="aops", bufs=3, space="PSUM") as aops, \
                 tc.tile_pool(name="l2ps", bufs=1, space="PSUM") as l2ps, \
                 tc.tile_pool(name="aot", bufs=3) as aot, \
                 tc.tile_pool(name="l2t", bufs=1) as l2t:
                sum_ps = l2ps.tile([1, TOK], FP, tag="l2sum")
                ssq_ps = l2ps.tile([1, TOK], FP, tag="l2ssq")

                def ao_chain(m):
                    ps = aops.tile([128, TOK], FP, tag="aops")
                    for j in range(4):
                        nc.tensor.matmul(
                            ps, waor[:, m, j],
                            orecv2[:, 2 * j:2 * j + 2, :], perf_mode=DR,
                            start=(j == 0), stop=(j == 3),
                            skip_group_check=True)
                    ao_sb = aot.tile([128, TOK], FP, tag="ao")
                    nc.scalar.copy(out=ao_sb, in_=ps)
                    nc.vector.scalar_tensor_tensor(
                        out=x2T[m], in0=ao_sb,
                        scalar=gmsa_sb[:, m:m + 1],
                        in1=xsT[:, m, :], op0=ALU.mult, op1=ALU.add)
                    nc.vector.tensor_copy(out=x2b[m], in_=x2T[m])
                    nc.vector.tensor_mul(sqb[m], x2b[m], x2b[m])

                def ln2_acc(m):
                    nc.tensor.matmul(sum_ps, ones_sb, x2b[m],
                                     start=(m == 0), stop=(m == 7),
                                     skip_group_check=True)
                    nc.tensor.matmul(ssq_ps, ones_sb, sqb[m],
                                     start=(m == 0), stop=(m == 7),
                                     skip_group_check=True)

                ao_chain(0)
                for m in range(1, 8):
                    ao_chain(m)
                    ln2_acc(m - 1)
                ln2_acc(7)

                mu2f = l2t.tile([1, TOK], FP)
                nc.vector.tensor_scalar_mul(mu2f, sum_ps, 1.0 / D)
                var2 = l2t.tile([1, TOK], FP)
                musq = l2t.tile([1, TOK], FP)
                nc.vector.tensor_mul(musq, mu2f, mu2f)
                nc.vector.tensor_scalar_mul(var2, ssq_ps, 1.0 / D)
                nc.vector.tensor_sub(var2, var2, musq)
                sd2 = l2t.tile([1, TOK], FP)
                nc.scalar.activation(out=sd2, in_=var2, func=AF.Sqrt,
                                     bias=eps1, scale=1.0)
                rstd2 = l2t.tile([1, TOK], FP)
                nc.vector.reciprocal_approx_fast(out=rstd2, in_=sd2)
                # row broadcasts (sources live at partition 0)
                mu2bc = l2t.tile([128, TOK], FP)
                nc.gpsimd.partition_broadcast(mu2bc, mu2f)
                rstd2bc = l2t.tile([128, TOK], FP)
                nc.gpsimd.partition_broadcast(rstd2bc, rstd2)
                for k in range(8):
                    u = l2t.tile([128, TOK], FP, tag="u", bufs=2)
                    nc.gpsimd.tensor_sub(u, x2T[k], mu2bc)
                    nc.vector.tensor_mul(h2dr[k // 2][:, k % 2, :],
                                         u, rstd2bc)

            if dbg_d is not None:
                nc.sync.dma_start(
                    out=dbgf8_d[1].rearrange("p (k t) -> p k t", t=TOK),
                    in_=orecv2)
                for m in range(8):
                    nc.sync.dma_start(
                        out=dbg32_d[1][:, TOK * m:TOK * m + TOK],
                        in_=x2T[m])
                for j in range(4):
                    nc.sync.dma_start(
                        out=dbgf8_d[2].rearrange(
                            "p (j i t) -> p j i t", j=4, i=2)[:, j],
                        in_=h2dr[j])

            # ---------------- phase 6: MLP (fp8 DoubleRow) -------------
            g_dr = gp.tile([128, 16, 2, TOK], F8, name="g_dr")
            with tc.tile_pool(name="m1ps", bufs=3, space="PSUM") as m1ps:
                for g in range(8):
                    for mi in range(4):
                        m = 4 * g + mi
                        ps = m1ps.tile([128, TOK], FP, tag="m1")
                        for j in range(4):
                            nc.tensor.matmul(ps, w1r[:, g, mi, j],
                                             h2dr[j], perf_mode=DR,
                                             start=(j == 0), stop=(j == 3))
                        gfunc = (AF.Identity if os.environ.get("DBG_NO_GELU")
                                 else AF.Gelu_apprx_tanh)
                        nc.scalar.activation(out=g_dr[:, m // 2, m % 2, :],
                                             in_=ps, func=gfunc,
                                             bias=b1_sb[:, m:m + 1],
                                             scale=1.0 / W1SCALE)

            outT = gp.tile([128, 8, TOK], FP, name="outT")
            with tc.tile_pool(name="m2ps", bufs=3, space="PSUM") as m2ps, \
                 tc.tile_pool(name="outp", bufs=3) as outp:
                for m in range(8):
                    ps = m2ps.tile([128, TOK], FP, tag="m2")
                    for j in range(16):
                        nc.tensor.matmul(ps, w2r[:, m, j], g_dr[:, j],
                                         perf_mode=DR,
                                         start=(j == 0), stop=(j == 15))
                    # psum = W2SCALE*(m - b2); evac: gmlp/W2SCALE * ps + gb2
                    mo = outp.tile([128, TOK], FP, tag="mo")
                    nc.scalar.activation(out=mo, in_=ps, func=AF.Identity,
                                         bias=gb2_sb[:, m:m + 1],
                                         scale=gmlp_sb[:, m:m + 1])
                    nc.vector.tensor_add(outT[:, m, :], mo, x2T[m])
                    nc.sync.dma_start(out=out_d[:, m, :],
                                      in_=outT[:, m, :])
            w2p.release()
            w1p.release()
            waop.release()

    nc.compile()
    return nc


# ---------------------------------------------------------------------------
# host side
# ---------------------------------------------------------------------------

_NC = None


def _get_nc():
    global _NC
    if _NC is None:
        _NC = build_program()
    return _NC


def _mask01_tiles():
    a = np.arange(128) // BS
    diag = (a[:, None] == a[None, :])
    strict = (a[None, :] > a[:, None])
    incl = (a[None, :] >= a[:, None])
    m = np.concatenate([diag, strict, incl],
                       axis=1).astype(np.float32)
    return np.ascontiguousarray(m.astype(bf16))


def _tile4(wT, km, mm):
    """[K, M] -> (m, p, k, c) with arr[m, p, k, c] = wT[128k+p, 128m+c]."""
    return wT.reshape(km, 128, mm, 128).transpose(2, 1, 0, 3)


def _prep_inputs(x, c, cos, sin, norm1_w, qkv_w, attn_out_w, norm2_w,
                 mlp_w1, mlp_b1, mlp_w2, mlp_b2, adaLN_w, adaLN_b):
    f32 = np.float32
    x = np.asarray(x, f32).reshape(S, D)
    c = np.asarray(c, f32).reshape(COND)
    cos = np.asarray(cos, f32)
    sin = np.asarray(sin, f32)
    qkv_w = np.asarray(qkv_w, f32)
    mlp_w1 = np.asarray(mlp_w1, f32)

    # adaLN modulation on host
    mods = adaLN_w.astype(f32) @ c + np.asarray(adaLN_b, f32)
    sh_msa, sc_msa, g_msa, sh_mlp, sc_mlp, g_mlp = mods.reshape(6, D)

    gam1 = (1.0 + sc_msa) * np.asarray(norm1_w, f32)          # [D]
    qkv_ws = qkv_w * gam1[None, :]                            # [3D, D]
    u_qkv = qkv_ws.sum(axis=1)                                # [3D]
    b_qkv = qkv_w @ sh_msa                                    # [3D]

    gam2 = (1.0 + sc_mlp) * np.asarray(norm2_w, f32)          # [D]
    w1s = mlp_w1 * gam2[None, :]                              # [4D, D]
    b1f = np.asarray(mlp_b1, f32) + mlp_w1 @ sh_mlp           # [4D]
    b2 = np.asarray(mlp_b2, f32)

    xb = x.astype(bf16)
    # LN1 stats on host (fp32)
    mu = x.mean(axis=1)
    sd = np.sqrt(x.var(axis=1) + 1e-5)
    rstd = 1.0 / sd
    rows2 = np.ascontiguousarray(np.stack([-mu, sd]).astype(bf16))
    r8row = np.ascontiguousarray((rstd * 0.125)[None, :].astype(bf16))
    rstdc = np.ascontiguousarray(rstd.reshape(16, 128).T.astype(f32))
    # qkv moving: (n, j, p, i, t) fp8
    xT8 = np.ascontiguousarray(
        x.T.reshape(4, 2, 128, 4, 512).transpose(3, 0, 2, 1, 4)
        .astype(fp8))
    xTb = xb.T.reshape(8, 128, S)                             # (k, p, t)

    # rope tables [2, 128, 1024]: cos | dest-signed sin; 1/WQSCALE folded.
    cs = np.concatenate([cos, cos], axis=-1).T                # [64, N]
    ss = np.concatenate([-sin.T, sin.T], axis=0)              # [64, N]
    cos128 = np.vstack([cs, cs])                              # [128, N]
    sin128 = np.vstack([ss, ss])
    trig = np.ascontiguousarray(
        (np.stack([cos128, sin128]) / WQSCALE).astype(bf16))

    # attn_out: (p=(i,hd), m, j, i2, c); f = (2*(2j+i2)+i)*64 + hd
    waoT = np.ascontiguousarray(
        np.clip(np.asarray(attn_out_w, f32).T * WAOSCALE, -240, 240)
        .reshape(4, 2, 2, 64, 8, 128).transpose(2, 3, 4, 0, 1, 5)
        .reshape(128, 8, 4, 2, 128).astype(fp8))
    # w1: (p, g, mi, j, i, c)
    w1q = np.clip(_tile4(w1s.T, 8, 32) * W1SCALE, -240, 240)  # [32,128,8,128]
    w1T = np.ascontiguousarray(
        w1q.reshape(8, 4, 128, 4, 2, 128).transpose(2, 0, 1, 3, 4, 5)
        .astype(fp8))                                 # (p, g, mi, j, i, c)
    # w2: (p, m, j, i, c)
    w2q = np.clip(_tile4(np.asarray(mlp_w2, f32).T, 32, 8) * W2SCALE,
                  -240, 240)                          # [8, 128, 32, 128]
    w2T = np.ascontiguousarray(
        w2q.reshape(8, 128, 16, 2, 128).transpose(1, 0, 2, 3, 4)
        .astype(fp8))

    smallc = np.ascontiguousarray(np.hstack([
        (g_msa / WAOSCALE).reshape(8, 128).T,
        (g_mlp / W2SCALE).reshape(8, 128).T,
        b1f.reshape(32, 128).T,
        b2.reshape(8, 128).T,
        (g_mlp * b2).reshape(8, 128).T]).astype(f32))         # [128, 64]

    common = {
        "rows2": rows2, "r8row": r8row, "rstdc": rstdc,
        "xT": xT8,
        "waoT": waoT, "w1T": w1T, "w2T": w2T,
        "smallc": smallc, "trig": trig,
        "mask01": _mask01_tiles(),
    }
    in_maps = []
    for j in range(NCORES):
        wq = np.stack([
            np.clip(
                qkv_ws[s * D + 128 * j: s * D + 128 * j + 128].T
                * WQSCALE, -240, 240)
            .reshape(4, 2, 128, 128).transpose(0, 2, 1, 3)
            for s in range(3)])  # [s, j, p, i, c]
        wq = np.ascontiguousarray(
            wq.transpose(2, 0, 1, 3, 4).astype(fp8))  # (p, s, j, i, c)
        ub = np.stack([
            np.concatenate([u_qkv[s * D + 128 * j: s * D + 128 * j + 128]
                            for s in range(3)]),
            np.concatenate([b_qkv[s * D + 128 * j: s * D + 128 * j + 128]
                            for s in range(3)])]) * WQSCALE  # [2, 384]
        m = dict(common)
        m["wqkvT"] = wq
        m["ubrow"] = np.ascontiguousarray(ub.astype(bf16))
        m["xsliceT"] = np.ascontiguousarray(
            xTb[:, :, TOK * j:TOK * j + TOK].transpose(1, 0, 2))  # (p,k,t)
        in_maps.append(m)
    return in_maps


def _assemble(res):
    """Gather per-core outputs [128, 8, TOK] (p, m, t) into [1, S, D]."""
    parts = []
    for j in range(NCORES):
        o = res.results[j]["out"]  # [128, 8, TOK]
        parts.append(np.ascontiguousarray(
            o.transpose(2, 1, 0).reshape(TOK, D)))
    return np.concatenate(parts, axis=0).reshape(1, S, D).astype(np.float32)


def kernel(**inputs):
    nc = _get_nc()
    in_maps = _prep_inputs(**inputs)
    res = run_bass_kernel_spmd(nc, in_maps, core_ids=list(range(NCORES)))
    return _assemble(res)
